# revision 2
# baseline (speedup 1.0000x reference)
"""Trainium2 Bass kernel for nn_CosineSimAug — v2 (bf16 + software-pipelined).

Reference computation per batch element:
  sim = cosine_sim(template_feats, search_feats)          (n1, n2)
  fusion = concat([sim, xyz, template_feats])             (260, n1, n2)
  x = relu(W1@fusion+b1); relu(W2@x+b2); relu(W3@x+b3)    (256, n1, n2)
  x = max over n1                                         (256, n2)
  x = relu(W4@x+b4); W5@x+b5                              (256, n2)

v2 strategy (vs the f32r baseline):
  - All heavy matmuls in bf16 (tolerance 2e-2 >> bf16 error ~3e-3). PE
    streams 1 col/cycle either way, but the win is keeping the PE
    *continuously* fed: stalls drop the tensor clock from 2.4GHz to
    ~1.2GHz (measured 216ns vs 675ns per 512-col matmul).
  - Chunk stages are software-pipelined on the PE queue:
    L1(c) | L2(c-1) | L3(c-2) so the ACT/DVE copies of stage outputs
    happen while the PE works on other stages.
  - Layer-3 relu+bias+max-over-n folded into a DVE pair-max + stt
    (running max starts at 0 which absorbs the relu).
  - Elementwise copy work split across ACT / DVE / GpSimd so no single
    engine exceeds the PE's per-chunk time.
"""

import sys

sys.path.insert(0, "/opt/trn_rl_repo")

import numpy as np
import ml_dtypes
import concourse.bacc as bacc
import concourse.mybir as mybir
from concourse.tile import TileContext
from concourse.bass_utils import run_bass_kernel_spmd

N_CORES = 8
B, F, N1, N2 = 32, 256, 64, 256
EPS = 1e-8
f32 = mybir.dt.float32
bf16 = mybir.dt.bfloat16
bfnp = ml_dtypes.bfloat16

NPAIRS = N1 // 2     # 32 chunks per batch, chunk t covers n in {t, t+32}
CHUNK = 2 * N2       # 512 positions per chunk
HALF = NPAIRS // 2   # chunks per sim3 half-tile

_CACHE = {}

Relu = mybir.ActivationFunctionType.Relu
ADD = None  # set below
MAX = None


def build(BB, reps=1):
    nc = bacc.Bacc()
    add, mx = mybir.AluOpType.add, mybir.AluOpType.max

    search = nc.dram_tensor("search", [BB, F, N2], bf16, kind="ExternalInput")
    templ = nc.dram_tensor("templ", [BB, F, N1], bf16, kind="ExternalInput")
    xyzc = nc.dram_tensor("xyzc", [BB, 4, N1], bf16, kind="ExternalInput")
    w1_0_rep = nc.dram_tensor("w1_0_rep", [1, HALF * 256], bf16, kind="ExternalInput")
    w1bt = nc.dram_tensor("w1bt", [128, 2, 256], bf16, kind="ExternalInput")
    w1ct = nc.dram_tensor("w1ct", [4, 256], bf16, kind="ExternalInput")
    w2t = nc.dram_tensor("w2t", [128, 2, 256], bf16, kind="ExternalInput")
    w3t = nc.dram_tensor("w3t", [128, 2, 256], bf16, kind="ExternalInput")
    w4t = nc.dram_tensor("w4t", [128, 2, 256], bf16, kind="ExternalInput")
    w5t = nc.dram_tensor("w5t", [128, 2, 256], bf16, kind="ExternalInput")
    biases = nc.dram_tensor("biases", [128, 8], f32, kind="ExternalInput")
    sim3_init = nc.dram_tensor("sim3_init", [3, HALF * CHUNK], bf16, kind="ExternalInput")
    out = nc.dram_tensor("out", [BB, F, N2], f32, kind="ExternalOutput")

    with TileContext(nc) as tc:
        with (
            tc.tile_pool(name="const", bufs=1) as cpool,
            tc.tile_pool(name="per_batch", bufs=2) as bpool,
            tc.tile_pool(name="acts", bufs=4) as apool,
            tc.tile_pool(name="mm", bufs=6, space="PSUM") as mmpool,
            tc.tile_pool(name="prep_ps", bufs=2, space="PSUM") as ppool,
        ):
            # ---- constants / weights (loaded once) ----
            w1bt_sb = cpool.tile([128, 2, 256], bf16, tag="w1bt")
            w1ct_sb = cpool.tile([4, 256], bf16, tag="w1ct")
            w2t_sb = cpool.tile([128, 2, 256], bf16, tag="w2t")
            w3t_sb = cpool.tile([128, 2, 256], bf16, tag="w3t")
            w4t_sb = cpool.tile([128, 2, 256], bf16, tag="w4t")
            w5t_sb = cpool.tile([128, 2, 256], bf16, tag="w5t")
            bias_sb = cpool.tile([128, 8], f32, tag="bias")
            ones_col = cpool.tile([128, 1], bf16, tag="ones")
            nc.sync.dma_start(w1bt_sb[:], w1bt[:, :, :])
            nc.sync.dma_start(w1ct_sb[:], w1ct[:, :])
            nc.sync.dma_start(w2t_sb[:], w2t[:, :, :])
            nc.sync.dma_start(w3t_sb[:], w3t[:, :, :])
            nc.sync.dma_start(w4t_sb[:], w4t[:, :, :])
            nc.sync.dma_start(w5t_sb[:], w5t[:, :, :])
            nc.sync.dma_start(bias_sb[:], biases[:, :])
            nc.vector.memset(ones_col[:], 1.0)

            def bcol(layer, half):  # layer: 0=b2,1=b3,2=b4,3=b5
                return bias_sb[:, layer * 2 + half : layer * 2 + half + 1]

            sim3_h = []
            blh_h = []
            for s in range(2):
                sim3 = cpool.tile([3, HALF * CHUNK], bf16, tag=f"sim3_{s}")
                nc.sync.dma_start(sim3[:, :], sim3_init[:, :])
                sim3_h.append(sim3)
                blh = cpool.tile([3, HALF * 256], bf16, tag=f"blh_{s}", name=f"blh_{s}")
                nc.sync.dma_start(blh[0:1, :], w1_0_rep[:, :])
                blh_h.append(blh)

            batch_list = [ib for _ in range(reps) for ib in range(BB)]
            PREP_AT = 10
            state = {}

            def emit_prep(j, i):
                """Input DMAs + norms + gram + sim + base + sim3/blh fills."""
                s_sb = bpool.tile([128, 2, N2], bf16, tag="s_sb")
                t_sb = bpool.tile([128, 2, N1], bf16, tag="t_sb")
                xy_sb = bpool.tile([4, N1], bf16, tag="xy_sb")
                nc.sync.dma_start(s_sb[:], search[i, :, :].rearrange("(k p) m -> p k m", p=128))
                nc.sync.dma_start(t_sb[:], templ[i, :, :].rearrange("(k p) n -> p k n", p=128))
                nc.sync.dma_start(xy_sb[:], xyzc[i, :, :])

                # norms (squares in bf16; sums accumulate in f32 PSUM)
                t2 = bpool.tile([128, 2, N1], bf16, tag="t2")
                s2 = bpool.tile([128, 2, N2], bf16, tag="s2")
                nc.vector.tensor_mul(t2[:], t_sb[:], t_sb[:])
                nc.vector.tensor_mul(s2[:], s_sb[:], s_sb[:])
                ones_bf = ones_col  # bf16 ones column
                sst = ppool.tile([N1, 1], f32, tag="pp")
                nc.tensor.matmul(sst[:], t2[:, 0, :], ones_bf[:], start=True, stop=False)
                nc.tensor.matmul(sst[:], t2[:, 1, :], ones_bf[:], start=False, stop=True)
                sss = ppool.tile([1, N2], f32, tag="pp")
                nc.tensor.matmul(sss[:], ones_bf[:], s2[:, 0, :], start=True, stop=False)
                nc.tensor.matmul(sss[:], ones_bf[:], s2[:, 1, :], start=False, stop=True)

                rnt = bpool.tile([N1, 1], f32, tag="rnt")
                nc.scalar.sqrt(rnt[:], sst[:])
                nc.vector.tensor_scalar_max(rnt[:], rnt[:], EPS)
                nc.vector.reciprocal(rnt[:], rnt[:])
                rns = bpool.tile([1, N2], f32, tag="rns")
                nc.scalar.sqrt(rns[:], sss[:])
                nc.vector.tensor_scalar_max(rns[:], rns[:], EPS)
                nc.vector.reciprocal(rns[:], rns[:])
                rns_b = bpool.tile([N1, N2], f32, tag="rns_b")
                nc.gpsimd.partition_broadcast(rns_b[:], rns[:])

                # gram + sim
                g_ps = ppool.tile([N1, N2], f32, tag="pp")
                nc.tensor.matmul(g_ps[:], t_sb[:, 0, :], s_sb[:, 0, :], start=True, stop=False)
                nc.tensor.matmul(g_ps[:], t_sb[:, 1, :], s_sb[:, 1, :], start=False, stop=True)
                sim_a = bpool.tile([N1, N2], f32, tag="sim_a")
                nc.vector.tensor_scalar_mul(sim_a[:], g_ps[:], rnt[:])
                sim_sb = bpool.tile([N1, N2], f32, tag="sim_sb")
                nc.vector.tensor_mul(sim_sb[:], sim_a[:], rns_b[:])

                # base_T = [t; xyz; 1]^T @ W1aug -> (n1, 256)
                base_ps = ppool.tile([N1, 256], f32, tag="pp")
                nc.tensor.matmul(base_ps[:], t_sb[:, 0, :], w1bt_sb[:, 0, :], start=True, stop=False)
                nc.tensor.matmul(base_ps[:], t_sb[:, 1, :], w1bt_sb[:, 1, :], start=False, stop=False)
                nc.tensor.matmul(base_ps[:], xy_sb[:], w1ct_sb[:], start=False, stop=True)
                base_sb = bpool.tile([N1, 256], f32, tag="base_sb")
                nc.vector.tensor_copy(base_sb[:], base_ps[:])

                # run2[h] accumulates max over chunks of RAW p3 (bias/relu are
                # applied after the loop; both commute with max). Cols 0:256
                # track n in 0..31, 256:512 track n in 32..63.
                run2 = bpool.tile([128, 2, CHUNK], bf16, tag="run2")
                nc.vector.memset(run2[:], -1e30)
                state[j] = (run2, sim_sb, base_sb)

            def emit_fill(j, s):
                """Rewrite the shared sim3/blh half s with batch j's values.
                Must be emitted after every batch-(j-1) read of that half."""
                _, sim_sb, base_sb = state[j]
                r0 = sim3_h[s][0:1, :].rearrange(
                    "p (t two m) -> p t two m", two=2, m=N2
                )
                nc.gpsimd.dma_start(
                    r0[:, :, 0:1, :], sim_sb[s * HALF : (s + 1) * HALF, :]
                )
                nc.gpsimd.dma_start(
                    r0[:, :, 1:2, :], sim_sb[32 + s * HALF : 32 + (s + 1) * HALF, :]
                )
                blh = blh_h[s]
                nc.gpsimd.dma_start(
                    blh[1:2, :].rearrange("p (t o) -> p t o", o=256),
                    base_sb[s * HALF : (s + 1) * HALF, :],
                )
                nc.gpsimd.dma_start(
                    blh[2:3, :].rearrange("p (t o) -> p t o", o=256),
                    base_sb[32 + s * HALF : 32 + (s + 1) * HALF, :],
                )

            for j, i in enumerate(batch_list):
                if j == 0:
                    emit_prep(0, i)
                    emit_fill(0, 0)
                    emit_fill(0, 1)
                run2 = state[j][0]
                x1_t = {}
                x2_t = {}

                def emit_L1(c):
                    s, tl = divmod(c, HALF)
                    sim3, blh = sim3_h[s], blh_h[s]
                    x1 = []
                    for h in range(2):
                        p1h = mmpool.tile([128, CHUNK], f32, tag="mm")
                        nc.tensor.matmul(
                            p1h[:],
                            blh[0:3, tl * 256 + h * 128 : tl * 256 + h * 128 + 128],
                            sim3[0:3, tl * CHUNK : (tl + 1) * CHUNK],
                            start=True,
                            stop=True,
                        )
                        x1h = apool.tile([128, CHUNK], bf16, tag=f"x1_{h}")
                        nc.scalar.activation(x1h[:], p1h[:], Relu)
                        x1.append(x1h)
                    x1_t[c] = x1

                def emit_L2(c):
                    x1 = x1_t.pop(c)
                    x2 = []
                    for h in range(2):
                        p2 = mmpool.tile([128, CHUNK], f32, tag="mm")
                        nc.tensor.matmul(
                            p2[:], w2t_sb[:, 0, h * 128 : h * 128 + 128], x1[0][:],
                            start=True, stop=False,
                        )
                        nc.tensor.matmul(
                            p2[:], w2t_sb[:, 1, h * 128 : h * 128 + 128], x1[1][:],
                            start=False, stop=True,
                        )
                        x2h = apool.tile([128, CHUNK], bf16, tag=f"x2_{h}")
                        # balance the relu+bias copies: h0 on ACT, h1 alternating
                        if h == 0 or (c % 2 == 0):
                            nc.scalar.activation(x2h[:], p2[:], Relu, bias=bcol(0, h))
                        else:
                            nc.vector.tensor_scalar(
                                x2h[:], p2[:], bcol(0, h), 0.0, op0=add, op1=mx
                            )
                        x2.append(x2h)
                    x2_t[c] = x2

                def emit_L3(c):
                    x2 = x2_t.pop(c)
                    for h in range(2):
                        p3 = mmpool.tile([128, CHUNK], f32, tag="mm")
                        nc.tensor.matmul(
                            p3[:], w3t_sb[:, 0, h * 128 : h * 128 + 128], x2[0][:],
                            start=True, stop=False,
                        )
                        nc.tensor.matmul(
                            p3[:], w3t_sb[:, 1, h * 128 : h * 128 + 128], x2[1][:],
                            start=False, stop=True,
                        )
                        nc.vector.tensor_max(run2[:, h, :], p3[:], run2[:, h, :])

                nxt = j + 1 < len(batch_list)
                for c in range(NPAIRS + 2):
                    if c == PREP_AT and nxt:
                        emit_prep(j + 1, batch_list[j + 1])
                    if c == HALF and nxt:
                        # all batch-j reads of half 0 (chunks 0..HALF-1) are
                        # emitted; safe to refill it with batch j+1 values
                        emit_fill(j + 1, 0)
                    if c == NPAIRS and nxt:
                        emit_fill(j + 1, 1)
                    if c < NPAIRS:
                        emit_L1(c)
                    if 1 <= c <= NPAIRS:
                        emit_L2(c - 1)
                    if 2 <= c <= NPAIRS + 1:
                        emit_L3(c - 2)
                state.pop(j)

                # fold run2 pairs + bias + relu -> running (bf16)
                running = bpool.tile([128, 2, N2], bf16, tag="running")
                for h in range(2):
                    mfold = bpool.tile([128, N2], f32, tag=f"mfold_{h}", name=f"mfold_{h}")
                    nc.vector.tensor_max(
                        mfold[:], run2[:, h, 0:N2], run2[:, h, N2:CHUNK]
                    )
                    nc.vector.tensor_scalar(
                        running[:, h, :], mfold[:], bcol(1, h), 0.0, op0=add, op1=mx
                    )

                # layers 4, 5
                x4 = bpool.tile([128, 2, N2], bf16, tag="x4")
                for h in range(2):
                    p4 = ppool.tile([128, N2], f32, tag="pp")
                    nc.tensor.matmul(
                        p4[:], w4t_sb[:, 0, h * 128 : h * 128 + 128], running[:, 0, :],
                        start=True, stop=False,
                    )
                    nc.tensor.matmul(
                        p4[:], w4t_sb[:, 1, h * 128 : h * 128 + 128], running[:, 1, :],
                        start=False, stop=True,
                    )
                    nc.vector.tensor_scalar(
                        x4[:, h, :], p4[:], bcol(2, h), 0.0, op0=add, op1=mx,
                    )
                out_sb = bpool.tile([128, 2, N2], f32, tag="out_sb")
                for h in range(2):
                    p5 = ppool.tile([128, N2], f32, tag="pp")
                    nc.tensor.matmul(
                        p5[:], w5t_sb[:, 0, h * 128 : h * 128 + 128], x4[:, 0, :],
                        start=True, stop=False,
                    )
                    nc.tensor.matmul(
                        p5[:], w5t_sb[:, 1, h * 128 : h * 128 + 128], x4[:, 1, :],
                        start=False, stop=True,
                    )
                    nc.vector.tensor_scalar_add(out_sb[:, h, :], p5[:], bcol(3, h))
                nc.sync.dma_start(
                    out[i, :, :].rearrange("(k p) m -> p k m", p=128), out_sb[:]
                )

    nc.compile()
    return nc


def _sim3_init():
    arr = np.zeros((3, HALF * CHUNK), np.float32)
    pat = arr.reshape(3, HALF, 2, N2)
    pat[1, :, 0, :] = 1.0
    pat[2, :, 1, :] = 1.0
    return arr.astype(bfnp)


def _prep_weights(W1, b1, W2, b2, W3, b3, W4, b4, W5, b5):
    def wt(W):  # out = W @ x ; lhsT layout [128p, 2k, 256o] with c = k*128+p
        return np.ascontiguousarray(
            W.T.reshape(2, 128, 256).transpose(1, 0, 2)
        ).astype(bfnp)

    return {
        "w1_0_rep": np.ascontiguousarray(
            np.tile(W1[:, 0], HALF)[None, :]
        ).astype(bfnp),
        "w1bt": wt(W1[:, 4:260]),
        "w1ct": np.ascontiguousarray(
            np.concatenate([W1[:, 1:4].T, b1[None, :]], 0)
        ).astype(bfnp),
        "w2t": wt(W2),
        "w3t": wt(W3),
        "w4t": wt(W4),
        "w5t": wt(W5),
        "biases": np.ascontiguousarray(
            np.stack([b2, b3, b4, b5], 0).reshape(4, 2, 128).transpose(2, 0, 1).reshape(128, 8),
            dtype=np.float32,
        ),
        "sim3_init": _sim3_init(),
    }


def _make_in_maps(search_feats, template_feats, template_seeds, wmaps, BB):
    xyzc_all = np.concatenate(
        [np.asarray(template_seeds).transpose(0, 2, 1), np.ones((B, 1, N1), np.float32)], 1
    ).astype(bfnp)
    search_feats = np.asarray(search_feats, dtype=np.float32).astype(bfnp)
    template_feats = np.asarray(template_feats, dtype=np.float32).astype(bfnp)
    in_maps = []
    for c in range(N_CORES):
        sl = slice(c * BB, (c + 1) * BB)
        m = dict(wmaps)
        m["search"] = np.ascontiguousarray(search_feats[sl])
        m["templ"] = np.ascontiguousarray(template_feats[sl])
        m["xyzc"] = np.ascontiguousarray(xyzc_all[sl])
        in_maps.append(m)
    return in_maps


def kernel(search_feats, template_feats, template_seeds,
           W1, b1, W2, b2, W3, b3, W4, b4, W5, b5):
    BB = B // N_CORES
    if "nc" not in _CACHE:
        _CACHE["nc"] = build(BB)
    nc = _CACHE["nc"]

    wmaps = _prep_weights(W1, b1, W2, b2, W3, b3, W4, b4, W5, b5)
    in_maps = _make_in_maps(search_feats, template_feats, template_seeds, wmaps, BB)
    res = run_bass_kernel_spmd(nc, in_maps, core_ids=list(range(N_CORES)))
    _CACHE["last_exec_ns"] = res.exec_time_ns
    return np.concatenate([res.results[c]["out"] for c in range(N_CORES)], 0)
